# revision 36
# baseline (speedup 1.0000x reference)
"""Trainium2 Bass kernel for per-query-pair attention (GNN message passing).

Math (reference):
  q = query @ Wq.T + bq                          [B,N,E]
  k = keys @ Wk.T + bk ; v = keys @ Wv.T + bv    [B,N,N,E]
  scores[b,h,i,j] = <k_h[b,i,j], q_h[b,i]> / sqrt(D); probs = softmax_j
  ctx[b,h,i,:]    = sum_j probs * v_h[b,i,j]

Key algebraic collapse (avoids projecting the 128MB keys tensor):
  scores[b,h,i,j] = <keys[b,i,j,:], qk[b,i,h,:]>  with
      qk[b,i,h,:]  = Wk_h.T @ (Wq_h @ query[b,i] + bq_h) / sqrt(D)   (tiny)
  bk drops out of softmax (constant over j).
  ctx[b,h,i,:]    = Wv_h @ u[b,i,h,:] + bv_h      with
      u[b,i,h,:]  = sum_j probs[b,h,i,j] * keys[b,i,j,:]
  (bv passes through since sum_j probs = 1.)

The big tensor is only ever contracted raw: once over e (scores) and once
over j (context weights). Keys ship in fp8 e3m4 (4 mantissa bits; keys are
N(0,1) so a x2 scale sits well inside e3m4's +-15.5 range) in both layouts
([j,i,e] natural and [e,i,j] transposed) - 8.4MB/core, half the bf16 cost,
measured rel-err ~1.7e-2 vs the 2e-2 gate. The x2 key scale is undone on
host: qk carries /2 (scores exact) and Wv carries /2 (context exact).

Per 16-query chunk: scores (4-query col-group packed matmuls) -> exp with
accum_out row sums (free Z) -> probs = Copy*1/Z on scalar -> PE transpose ->
u matmuls (fp8 stationary, FWL) -> per-chunk Wv projection into a transposed
f32 accumulator. Tail after the last DMA byte is just one chunk's u+proj plus
two 128x128 transposes and the output DMA.

Sharding: data-parallel over B (8 batches over 8 cores), zero collectives.
"""

import math

import numpy as np
import ml_dtypes

B, N, E, H, D = 8, 128, 256, 8, 32
NCORES = 8
NCH = 8                      # chunks of 16 queries
GC = 16
NG4 = GC // 4                # score groups (of 4 queries) per chunk
BF16 = ml_dtypes.bfloat16
F8E3 = ml_dtypes.float8_e3m4
KSCALE = np.float32(2.0)     # keys quantize as 2k in e3m4; qk and Wv carry /2

_CACHE = {}


def _enable_ldw_opt():
    """Flip walrus's hardcoded --enable-ldw-opt=false to true (fast weight
    load) when CC_LDW_OPT=1, for A/B testing the u-matmul weight-load rate."""
    import os

    if os.environ.get("CC_LDW_OPT", "0") != "1":
        return
    from concourse import bass_utils

    if getattr(bass_utils, "_ldw_patched", False):
        return
    orig = bass_utils.run_command

    def patched(argv, **kw):
        argv = [
            "--enable-ldw-opt=true" if a == "--enable-ldw-opt=false" else a
            for a in argv
        ]
        return orig(argv, **kw)

    bass_utils.run_command = patched
    bass_utils._ldw_patched = True


def _build_bass():
    import concourse.bass as bass  # noqa: F401
    import concourse.mybir as mybir
    from concourse import bacc
    import concourse.tile as tile
    from concourse.masks import make_identity

    dt = mybir.dt
    fp32 = dt.float32
    bf16 = dt.bfloat16
    fp8 = dt.float8e3

    nc = bacc.Bacc()

    # [j, i, e] fp8 - natural layout, j on partitions
    ks_nat = nc.declare_dram_parameter("ks_nat", [N, N, E], fp8, isOutput=False)
    # [half, e_half, i, j] fp8 - transposed layout, e on partitions
    ks_t = nc.declare_dram_parameter("ks_t", [2, 128, N, N], fp8, isOutput=False)
    # [half, e_half, i, h] bf16 - per-query qk vectors (carry the /2 key scale)
    qk = nc.declare_dram_parameter("qk", [2, 128, N, H], bf16, isOutput=False)
    # [half, e_half, e_out] bf16 - Wv.T / 2
    wvt = nc.declare_dram_parameter("wvt", [2, 128, E], bf16, isOutput=False)
    # [p, half] f32 - bv rearranged so partition p = e_out % 128
    bvp = nc.declare_dram_parameter("bvp", [128, 2], fp32, isOutput=False)
    out = nc.declare_dram_parameter("out", [N, E], fp32, isOutput=True)

    with tile.TileContext(nc) as tc:
        with (
            tc.tile_pool(name="const", bufs=1) as const,
            tc.tile_pool(name="ksn", bufs=1) as ksn_pool,
            tc.tile_pool(name="kst", bufs=1) as kst_pool,
            tc.tile_pool(name="work", bufs=5) as work,
            tc.tile_pool(name="ps_sc", bufs=3, space="PSUM") as ps_sc,
            tc.tile_pool(name="ps_pt", bufs=2, space="PSUM") as ps_pt,
            tc.tile_pool(name="ps_u", bufs=2, space="PSUM") as ps_u,
            tc.tile_pool(name="ps_pj", bufs=1, space="PSUM") as ps_pj,
        ):
            ident_bf = const.tile([128, 128], bf16, tag="ident_bf")
            make_identity(nc, ident_bf)
            ident_f32 = const.tile([128, 128], fp32, tag="ident_f32")
            make_identity(nc, ident_f32)
            # transposed output accumulator [e_out%128, hg, i] f32
            osbT = const.tile([128, 2, N], fp32, tag="osbT")

            # qk for all queries upfront on the sync ring (chunk-0 scores
            # need it); keys stream per-chunk on two rings: sync=ksn,
            # scalar=kst. wvt/bv ride the lighter scalar ring early.
            # ONE DMA ring (sync) for all key data, issued up front in exact
            # consumption order: kst_c right before ksn_c, chunk by chunk.
            # Two competing rings share the 16 DMA engines in rough enqueue
            # order, so whichever ring queues first starves the other for
            # milliseconds-relevant stretches (measured: kst crawled at
            # 43-92GB/s behind a flooded ksn ring while scores starved).
            # A single ring sustains ~330GB/s and completes granules in the
            # exact order compute consumes them. Sem-lane recycle waits on
            # the 17th+ issue stall only the sync FIFO, which has no compute.
            # wvt/bv ride scalar before any compute is queued there.
            wvt_sb = const.tile([128, 2, E], bf16, tag="wvt_sb")
            nc.scalar.dma_start(out=wvt_sb, in_=wvt.rearrange("h e o -> e h o"))
            bv_sb = const.tile([128, 2], fp32, tag="bv_sb")
            nc.scalar.dma_start(out=bv_sb, in_=bvp[:, :])
            # qk split: chunk 0's slice first so its scores start ~2us
            # earlier; the rest follows behind chunk 0's keys.
            qk_sb = const.tile([128, 2, N, H], bf16, tag="qk_sb")
            qk_r = qk.rearrange("h e i k -> e h i k")
            nc.sync.dma_start(
                out=qk_sb[:, :, : 2 * GC, :], in_=qk_r[:, :, : 2 * GC, :]
            )

            kst_tiles = []
            ksn_tiles = []
            for c in range(NCH):
                i0 = c * GC
                kst = kst_pool.tile([128, 2, GC, N], fp8, tag=f"kst{c}")
                nc.sync.dma_start(
                    out=kst,
                    in_=ks_t[:, :, i0 : i0 + GC, :].rearrange("h e i j -> e h i j"),
                )
                kst_tiles.append(kst)
                ksn = ksn_pool.tile([128, GC, E], fp8, tag=f"ksn{c}")
                nc.sync.dma_start(out=ksn, in_=ks_nat[:, i0 : i0 + GC, :])
                ksn_tiles.append(ksn)
                if c == 0:
                    nc.sync.dma_start(
                        out=qk_sb[:, :, 2 * GC :, :], in_=qk_r[:, :, 2 * GC :, :]
                    )

            # zero-padded qk stationaries: score matmuls then write all 32
            # rows of each psum strip (pads get exact zeros), so no per-chunk
            # sc memset is needed. Two tiles so chunk 0 only depends on its
            # own slice (tile deps are whole-tile); memsets run on DVE during
            # the startup DMA wait, the big copy is emitted later (inside the
            # loop) so it can't delay chunk 0/1's softmax on the DVE FIFO.
            qkp0 = const.tile([128, 2, 2 * GC, 32], bf16, tag="qkp0")
            qkpr = const.tile([128, 2, N - 2 * GC, 32], bf16, tag="qkpr")
            nc.vector.memset(qkp0, 0.0)
            nc.vector.memset(qkpr, 0.0)
            nc.vector.tensor_copy(qkp0[:, :, :, :H], qk_sb[:, :, : 2 * GC, :])

            def qkp_slice(half, i):
                if i < 2 * GC:
                    return qkp0[:, half, i, :]
                return qkpr[:, half, i - 2 * GC, :]

            def front(c):
                """Scores + softmax for chunk c. Returns state for back()."""
                i0 = c * GC
                kst = kst_tiles[c]

                # scores: sc[32*gi+h, g4, j] = sum_e qk[e,i,h] * kst[e,i,j]
                # stationary is the 32-col zero-padded qkp, so each matmul
                # writes its full 32-row strip (pads = exact zeros)
                sc = ps_sc.tile([128, NG4, N], fp32, tag="sc")
                for g4 in range(NG4):
                    for gi in range(4):
                        il = g4 * 4 + gi
                        for half in range(2):
                            nc.tensor.matmul(
                                sc[32 * gi : 32 * gi + 32, g4, :],
                                lhsT=qkp_slice(half, i0 + il),
                                rhs=kst[:, half, il, :],
                                start=(half == 0),
                                stop=(half == 1),
                                tile_position=(0, 32 * gi),
                            )

                # softmax over j (no max-sub: |scores| < ~6); exp on scalar,
                # everything else on DVE. The scalar FIFO otherwise only
                # holds PE-gated copies from back(), two chunks behind, so
                # exp never queues behind a stalled instruction.
                wsb = work.tile([128, NG4, N], bf16, tag="wsb")
                nc.scalar.activation(
                    out=wsb, in_=sc, func=mybir.ActivationFunctionType.Exp
                )
                zsb = work.tile([128, NG4], fp32, tag="zsb")
                nc.vector.reduce_sum(zsb, wsb, axis=mybir.AxisListType.X)
                rz = work.tile([128, NG4], fp32, tag="rz")
                nc.vector.reciprocal(rz, zsb)
                probs = work.tile([128, NG4, N], bf16, tag="probs")
                for g4 in range(NG4):
                    nc.vector.tensor_scalar_mul(
                        probs[:, g4, :], wsb[:, g4, :], rz[:, g4 : g4 + 1]
                    )
                return c, probs

            def back_t(state):
                """transpose probs -> [j, (gi, h)] and copy to SBUF."""
                c, probs = state
                pt = ps_pt.tile([128, NG4, 128], bf16, tag="pt")
                for g4 in range(NG4):
                    nc.tensor.transpose(pt[:, g4, :], probs[:, g4, :], ident_bf)
                ptsb = work.tile([128, NG4, 128], bf16, tag="ptsb")
                nc.vector.tensor_copy(ptsb, pt)
                return c, ptsb

            def back_u(state):
                """u + projection for a chunk whose probsT landed in SBUF.

                Runs one chunk behind back_t (and two behind front) so the
                PE's in-order queue never waits on the transpose->SBUF copy
                round trip: by the time u's matmuls pop, ptsb has been in
                SBUF for a full pipeline stage.
                """
                c, ptsb = state
                i0 = c * GC
                ksn = ksn_tiles[c]

                # u[e, half, i, h] = sum_j ksn[j, i, e] * probsT[j, (i,h)]
                ups = ps_u.tile([128, 2, GC, H], fp32, tag="ups")
                for g4 in range(NG4):
                    for gi in range(4):
                        il = g4 * 4 + gi
                        for half in range(2):
                            nc.tensor.matmul(
                                ups[:, half, il, :],
                                lhsT=ksn[:, il, 128 * half : 128 * (half + 1)],
                                rhs=ptsb[:, g4, 32 * gi : 32 * gi + H],
                                start=True,
                                stop=True,
                            )
                uc = work.tile([128, 2, GC, H], bf16, tag="uc")
                nc.scalar.copy(out=uc, in_=ups)

                # per-chunk Wv projection into osbT (hides the tail)
                # cps[32*hh+d, i] = sum_e Wv[h*32+d, e]/2 * u[e, i, h]
                # bias is added once at the very end (osbT has e_out on
                # partitions), keeping this copy off the DVE critical path
                for hg in range(2):
                    # bank-sized (512 f32 = 2KB) so the accumulation groups
                    # own their psum zero region exclusively
                    cpsb = ps_pj.tile([128, 512], fp32, tag="pj")
                    cps = cpsb[:, :GC]
                    for hh in range(4):
                        h = hg * 4 + hh
                        for half in range(2):
                            nc.tensor.matmul(
                                cps[32 * hh : 32 * hh + 32, :],
                                lhsT=wvt_sb[:, half, 32 * h : 32 * (h + 1)],
                                rhs=uc[:, half, :, h],
                                start=(half == 0),
                                stop=(half == 1),
                                tile_position=(0, 32 * hh),
                            )
                    nc.scalar.copy(out=osbT[:, hg, i0 : i0 + GC], in_=cps)

            # three-stage software pipeline, each stage chunks deeper:
            # u+proj(c-4) | scores(c) | transposes(c-2). back_u is emitted
            # FIRST in each iteration so uc/proj never queue on the scalar
            # FIFO behind a future chunk's exp; the PE always has future
            # scores queued while a softmax completes, and consumes each
            # ptsb well after the DVE copied it.
            fr, tr = [], []
            for c in range(NCH):
                if c == 2:
                    # fill the rest of the padded stationaries now: late
                    # enough not to delay chunk 0/1 softmax on the DVE FIFO,
                    # early enough for chunk 2's scores
                    nc.vector.tensor_copy(
                        qkpr[:, :, :, :H], qk_sb[:, :, 2 * GC :, :]
                    )
                if tr:
                    back_u(tr.pop(0))
                fr.append(front(c))
                if len(fr) > 2:
                    tr.append(back_t(fr.pop(0)))
            while fr:
                if tr:
                    back_u(tr.pop(0))
                tr.append(back_t(fr.pop(0)))
            while tr:
                back_u(tr.pop(0))

            # ---- tail: add bias (per-partition on osbT), transpose
            # [e_out, i] -> [i, e_out] and write out ----
            osb = const.tile([128, E], fp32, tag="osb")
            for hg in range(2):
                nc.vector.tensor_scalar_add(
                    osbT[:, hg, :], osbT[:, hg, :], bv_sb[:, hg : hg + 1]
                )
                ops = ps_pj.tile([128, 128], fp32, tag="pj")
                nc.tensor.transpose(ops, osbT[:, hg, :], ident_f32)
                nc.vector.tensor_copy(osb[:, 128 * hg : 128 * (hg + 1)], ops)

            nc.sync.dma_start(out=out[:, :], in_=osb)

    nc.finalize()
    return nc


def _host_prep(query_states, key_states, Wq, bq, Wk, bk, Wv, bv):
    """Build per-core input maps. bk is softmax-invariant and dropped."""
    f32 = np.float32
    qs = np.asarray(query_states, f32)
    ks = np.asarray(key_states, f32)
    Wq = np.asarray(Wq, f32)
    bq = np.asarray(bq, f32)
    Wk = np.asarray(Wk, f32)
    Wv = np.asarray(Wv, f32)
    bv = np.asarray(bv, f32)

    q = qs @ Wq.T + bq                                   # [B,N,E]
    qk = np.einsum(
        "bihd,hde->bihe", q.reshape(B, N, H, D), Wk.reshape(H, D, E)
    ) * f32(1.0 / math.sqrt(D) / KSCALE)                 # [B,N,H,E]

    wvt_host = np.ascontiguousarray((Wv.T / KSCALE).reshape(2, 128, E)).astype(BF16)
    bv_host = np.ascontiguousarray(bv.reshape(2, 128).T)

    in_maps = []
    for b in range(B):
        ksb = ks[b] * KSCALE
        in_maps.append(
            {
                "ks_nat": np.ascontiguousarray(ksb.transpose(1, 0, 2)).astype(F8E3),
                "ks_t": np.ascontiguousarray(ksb.transpose(2, 0, 1))
                .reshape(2, 128, N, N)
                .astype(F8E3),
                "qk": np.ascontiguousarray(qk[b].transpose(2, 0, 1))
                .reshape(2, 128, N, H)
                .astype(BF16),
                "wvt": wvt_host,
                "bvp": bv_host,
            }
        )
    return in_maps


def kernel(**inputs):
    _enable_ldw_opt()
    from concourse.bass_utils import run_bass_kernel_spmd

    if "nc" not in _CACHE:
        _CACHE["nc"] = _build_bass()
    nc = _CACHE["nc"]

    in_maps = _host_prep(**inputs)
    res = run_bass_kernel_spmd(nc, in_maps, core_ids=list(range(NCORES)))
    out = np.stack([r["out"] for r in res.results], axis=0)  # [B, N, E]
    return out.astype(np.float32)


# revision 37
# speedup vs baseline: 1.1329x; 1.1329x over previous
"""Trainium2 Bass kernel for per-query-pair attention (GNN message passing).

Math (reference):
  q = query @ Wq.T + bq                          [B,N,E]
  k = keys @ Wk.T + bk ; v = keys @ Wv.T + bv    [B,N,N,E]
  scores[b,h,i,j] = <k_h[b,i,j], q_h[b,i]> / sqrt(D); probs = softmax_j
  ctx[b,h,i,:]    = sum_j probs * v_h[b,i,j]

Key algebraic collapse (avoids projecting the 128MB keys tensor):
  scores[b,h,i,j] = <keys[b,i,j,:], qk[b,i,h,:]>  with
      qk[b,i,h,:]  = Wk_h.T @ (Wq_h @ query[b,i] + bq_h) / sqrt(D)   (tiny)
  bk drops out of softmax (constant over j).
  ctx[b,h,i,:]    = Wv_h @ u[b,i,h,:] + bv_h      with
      u[b,i,h,:]  = sum_j probs[b,h,i,j] * keys[b,i,j,:]
  (bv passes through since sum_j probs = 1.)

The big tensor is only ever contracted raw: once over e (scores) and once
over j (context weights). Keys ship in fp8 e3m4 (4 mantissa bits; keys are
N(0,1) so a x2 scale sits well inside e3m4's +-15.5 range) in both layouts
([j,i,e] natural and [e,i,j] transposed) - 8.4MB/core, half the bf16 cost,
measured rel-err ~1.7e-2 vs the 2e-2 gate. The x2 key scale is undone on
host: qk carries /2 (scores exact) and Wv carries /2 (context exact).

Per 16-query chunk: scores (4-query col-group packed matmuls) -> exp with
accum_out row sums (free Z) -> probs = Copy*1/Z on scalar -> PE transpose ->
u matmuls (fp8 stationary, FWL) -> per-chunk Wv projection into a transposed
f32 accumulator. Tail after the last DMA byte is just one chunk's u+proj plus
two 128x128 transposes and the output DMA.

Sharding: data-parallel over B (8 batches over 8 cores), zero collectives.
"""

import math

import numpy as np
import ml_dtypes

B, N, E, H, D = 8, 128, 256, 8, 32
NCORES = 8
NCH = 8                      # chunks of 16 queries
GC = 16
NG4 = GC // 4                # score groups (of 4 queries) per chunk
BF16 = ml_dtypes.bfloat16
F8E3 = ml_dtypes.float8_e3m4
KSCALE = np.float32(2.0)     # keys quantize as 2k in e3m4; qk and Wv carry /2

_CACHE = {}


def _enable_ldw_opt():
    """Flip walrus's hardcoded --enable-ldw-opt=false to true (fast weight
    load) when CC_LDW_OPT=1, for A/B testing the u-matmul weight-load rate."""
    import os

    if os.environ.get("CC_LDW_OPT", "0") != "1":
        return
    from concourse import bass_utils

    if getattr(bass_utils, "_ldw_patched", False):
        return
    orig = bass_utils.run_command

    def patched(argv, **kw):
        argv = [
            "--enable-ldw-opt=true" if a == "--enable-ldw-opt=false" else a
            for a in argv
        ]
        return orig(argv, **kw)

    bass_utils.run_command = patched
    bass_utils._ldw_patched = True


def _build_bass():
    import concourse.bass as bass  # noqa: F401
    import concourse.mybir as mybir
    from concourse import bacc
    import concourse.tile as tile
    from concourse.masks import make_identity

    dt = mybir.dt
    fp32 = dt.float32
    bf16 = dt.bfloat16
    fp8 = dt.float8e3

    nc = bacc.Bacc()

    # [j, i, e] fp8 - natural layout, j on partitions
    ks_nat = nc.declare_dram_parameter("ks_nat", [N, N, E], fp8, isOutput=False)
    # [half, e_half, i, j] fp8 - transposed layout, e on partitions
    ks_t = nc.declare_dram_parameter("ks_t", [2, 128, N, N], fp8, isOutput=False)
    # [half, e_half, i, h] bf16 - per-query qk vectors (carry the /2 key scale)
    qk = nc.declare_dram_parameter("qk", [2, 128, N, H], bf16, isOutput=False)
    # [half, e_half, e_out] bf16 - Wv.T / 2
    wvt = nc.declare_dram_parameter("wvt", [2, 128, E], bf16, isOutput=False)
    # [p, half] f32 - bv rearranged so partition p = e_out % 128
    bvp = nc.declare_dram_parameter("bvp", [128, 2], fp32, isOutput=False)
    out = nc.declare_dram_parameter("out", [N, E], fp32, isOutput=True)

    with tile.TileContext(nc) as tc:
        with (
            tc.tile_pool(name="const", bufs=1) as const,
            tc.tile_pool(name="ksn", bufs=1) as ksn_pool,
            tc.tile_pool(name="kst", bufs=1) as kst_pool,
            tc.tile_pool(name="work", bufs=5) as work,
            tc.tile_pool(name="ps_sc", bufs=3, space="PSUM") as ps_sc,
            tc.tile_pool(name="ps_pt", bufs=2, space="PSUM") as ps_pt,
            tc.tile_pool(name="ps_u", bufs=2, space="PSUM") as ps_u,
            tc.tile_pool(name="ps_pj", bufs=1, space="PSUM") as ps_pj,
        ):
            ident_bf = const.tile([128, 128], bf16, tag="ident_bf")
            make_identity(nc, ident_bf)
            ident_f32 = const.tile([128, 128], fp32, tag="ident_f32")
            make_identity(nc, ident_f32)
            # transposed output accumulator [e_out%128, hg, i] f32
            osbT = const.tile([128, 2, N], fp32, tag="osbT")

            # qk for all queries upfront on the sync ring (chunk-0 scores
            # need it); keys stream per-chunk on two rings: sync=ksn,
            # scalar=kst. wvt/bv ride the lighter scalar ring early.
            # ONE DMA ring (sync) for all key data, issued up front in exact
            # consumption order: kst_c right before ksn_c, chunk by chunk.
            # Two competing rings share the 16 DMA engines in rough enqueue
            # order, so whichever ring queues first starves the other for
            # milliseconds-relevant stretches (measured: kst crawled at
            # 43-92GB/s behind a flooded ksn ring while scores starved).
            # A single ring sustains ~330GB/s and completes granules in the
            # exact order compute consumes them. Sem-lane recycle waits on
            # the 17th+ issue stall only the sync FIFO, which has no compute.
            # wvt/bv ride scalar before any compute is queued there.
            wvt_sb = const.tile([128, 2, E], bf16, tag="wvt_sb")
            nc.scalar.dma_start(out=wvt_sb, in_=wvt.rearrange("h e o -> e h o"))
            bv_sb = const.tile([128, 2], fp32, tag="bv_sb")
            nc.scalar.dma_start(out=bv_sb, in_=bvp[:, :])
            qk_sb = const.tile([128, 2, N, H], bf16, tag="qk_sb")
            nc.sync.dma_start(out=qk_sb, in_=qk.rearrange("h e i k -> e h i k"))

            kst_tiles = []
            ksn_tiles = []
            for c in range(NCH):
                i0 = c * GC
                kst = kst_pool.tile([128, 2, GC, N], fp8, tag=f"kst{c}")
                nc.sync.dma_start(
                    out=kst,
                    in_=ks_t[:, :, i0 : i0 + GC, :].rearrange("h e i j -> e h i j"),
                )
                kst_tiles.append(kst)
                ksn = ksn_pool.tile([128, GC, E], fp8, tag=f"ksn{c}")
                nc.sync.dma_start(out=ksn, in_=ks_nat[:, i0 : i0 + GC, :])
                ksn_tiles.append(ksn)

            def front(c):
                """Scores + softmax for chunk c. Returns state for back()."""
                i0 = c * GC
                kst = kst_tiles[c]

                # scores: sc[32*gi+h, g4, j] = sum_e qk[e,i,h] * kst[e,i,j]
                # matmuls write 8 of every 32 rows; zero first so the batched
                # softmax reads only finite values
                sc = ps_sc.tile([128, NG4, N], fp32, tag="sc")
                nc.vector.memset(sc, 0.0)
                for g4 in range(NG4):
                    for gi in range(4):
                        il = g4 * 4 + gi
                        for half in range(2):
                            nc.tensor.matmul(
                                sc[32 * gi : 32 * gi + H, g4, :],
                                lhsT=qk_sb[:, half, i0 + il, :],
                                rhs=kst[:, half, il, :],
                                start=(half == 0),
                                stop=(half == 1),
                                tile_position=(0, 32 * gi),
                            )

                # softmax over j (no max-sub: |scores| < ~6); exp on scalar,
                # everything else on DVE. The scalar FIFO otherwise only
                # holds PE-gated copies from back(), two chunks behind, so
                # exp never queues behind a stalled instruction.
                wsb = work.tile([128, NG4, N], bf16, tag="wsb")
                nc.scalar.activation(
                    out=wsb, in_=sc, func=mybir.ActivationFunctionType.Exp
                )
                zsb = work.tile([128, NG4], fp32, tag="zsb")
                nc.vector.reduce_sum(zsb, wsb, axis=mybir.AxisListType.X)
                rz = work.tile([128, NG4], fp32, tag="rz")
                nc.vector.reciprocal(rz, zsb)
                probs = work.tile([128, NG4, N], bf16, tag="probs")
                for g4 in range(NG4):
                    nc.vector.tensor_scalar_mul(
                        probs[:, g4, :], wsb[:, g4, :], rz[:, g4 : g4 + 1]
                    )
                return c, probs

            def back_t(state):
                """transpose probs -> [j, (gi, h)] and copy to SBUF."""
                c, probs = state
                pt = ps_pt.tile([128, NG4, 128], bf16, tag="pt")
                for g4 in range(NG4):
                    nc.tensor.transpose(pt[:, g4, :], probs[:, g4, :], ident_bf)
                ptsb = work.tile([128, NG4, 128], bf16, tag="ptsb")
                nc.vector.tensor_copy(ptsb, pt)
                return c, ptsb

            def back_u(state):
                """u + projection for a chunk whose probsT landed in SBUF.

                Runs one chunk behind back_t (and two behind front) so the
                PE's in-order queue never waits on the transpose->SBUF copy
                round trip: by the time u's matmuls pop, ptsb has been in
                SBUF for a full pipeline stage.
                """
                c, ptsb = state
                i0 = c * GC
                ksn = ksn_tiles[c]

                # u[e, half, i, h] = sum_j ksn[j, i, e] * probsT[j, (i,h)]
                ups = ps_u.tile([128, 2, GC, H], fp32, tag="ups")
                for g4 in range(NG4):
                    for gi in range(4):
                        il = g4 * 4 + gi
                        for half in range(2):
                            nc.tensor.matmul(
                                ups[:, half, il, :],
                                lhsT=ksn[:, il, 128 * half : 128 * (half + 1)],
                                rhs=ptsb[:, g4, 32 * gi : 32 * gi + H],
                                start=True,
                                stop=True,
                            )
                uc = work.tile([128, 2, GC, H], bf16, tag="uc")
                nc.scalar.copy(out=uc, in_=ups)

                # per-chunk Wv projection into osbT (hides the tail)
                # cps[32*hh+d, i] = sum_e Wv[h*32+d, e]/2 * u[e, i, h]
                # bias is added once at the very end (osbT has e_out on
                # partitions), keeping this copy off the DVE critical path
                for hg in range(2):
                    # bank-sized (512 f32 = 2KB) so the accumulation groups
                    # own their psum zero region exclusively
                    cpsb = ps_pj.tile([128, 512], fp32, tag="pj")
                    cps = cpsb[:, :GC]
                    for hh in range(4):
                        h = hg * 4 + hh
                        for half in range(2):
                            nc.tensor.matmul(
                                cps[32 * hh : 32 * hh + 32, :],
                                lhsT=wvt_sb[:, half, 32 * h : 32 * (h + 1)],
                                rhs=uc[:, half, :, h],
                                start=(half == 0),
                                stop=(half == 1),
                                tile_position=(0, 32 * hh),
                            )
                    nc.scalar.copy(out=osbT[:, hg, i0 : i0 + GC], in_=cps)

            # three-stage software pipeline, each stage chunks deeper:
            # u+proj(c-4) | scores(c) | transposes(c-2). back_u is emitted
            # FIRST in each iteration so uc/proj never queue on the scalar
            # FIFO behind a future chunk's exp; the PE always has future
            # scores queued while a softmax completes, and consumes each
            # ptsb well after the DVE copied it.
            fr, tr = [], []
            for c in range(NCH):
                fr.append(front(c))
                if len(fr) > 2:
                    tr.append(back_t(fr.pop(0)))
                if len(tr) > 1:
                    back_u(tr.pop(0))
            while fr:
                tr.append(back_t(fr.pop(0)))
                if len(tr) > 1:
                    back_u(tr.pop(0))
            while tr:
                back_u(tr.pop(0))

            # ---- tail: add bias (per-partition on osbT), transpose
            # [e_out, i] -> [i, e_out] and write out ----
            osb = const.tile([128, E], fp32, tag="osb")
            for hg in range(2):
                nc.vector.tensor_scalar_add(
                    osbT[:, hg, :], osbT[:, hg, :], bv_sb[:, hg : hg + 1]
                )
                ops = ps_pj.tile([128, 128], fp32, tag="pj")
                nc.tensor.transpose(ops, osbT[:, hg, :], ident_f32)
                nc.vector.tensor_copy(osb[:, 128 * hg : 128 * (hg + 1)], ops)

            nc.sync.dma_start(out=out[:, :], in_=osb)

    nc.finalize()
    return nc


def _host_prep(query_states, key_states, Wq, bq, Wk, bk, Wv, bv):
    """Build per-core input maps. bk is softmax-invariant and dropped."""
    f32 = np.float32
    qs = np.asarray(query_states, f32)
    ks = np.asarray(key_states, f32)
    Wq = np.asarray(Wq, f32)
    bq = np.asarray(bq, f32)
    Wk = np.asarray(Wk, f32)
    Wv = np.asarray(Wv, f32)
    bv = np.asarray(bv, f32)

    q = qs @ Wq.T + bq                                   # [B,N,E]
    qk = np.einsum(
        "bihd,hde->bihe", q.reshape(B, N, H, D), Wk.reshape(H, D, E)
    ) * f32(1.0 / math.sqrt(D) / KSCALE)                 # [B,N,H,E]

    wvt_host = np.ascontiguousarray((Wv.T / KSCALE).reshape(2, 128, E)).astype(BF16)
    bv_host = np.ascontiguousarray(bv.reshape(2, 128).T)

    in_maps = []
    for b in range(B):
        ksb = ks[b] * KSCALE
        in_maps.append(
            {
                "ks_nat": np.ascontiguousarray(ksb.transpose(1, 0, 2)).astype(F8E3),
                "ks_t": np.ascontiguousarray(ksb.transpose(2, 0, 1))
                .reshape(2, 128, N, N)
                .astype(F8E3),
                "qk": np.ascontiguousarray(qk[b].transpose(2, 0, 1))
                .reshape(2, 128, N, H)
                .astype(BF16),
                "wvt": wvt_host,
                "bvp": bv_host,
            }
        )
    return in_maps


def kernel(**inputs):
    _enable_ldw_opt()
    from concourse.bass_utils import run_bass_kernel_spmd

    if "nc" not in _CACHE:
        _CACHE["nc"] = _build_bass()
    nc = _CACHE["nc"]

    in_maps = _host_prep(**inputs)
    res = run_bass_kernel_spmd(nc, in_maps, core_ids=list(range(NCORES)))
    out = np.stack([r["out"] for r in res.results], axis=0)  # [B, N, E]
    return out.astype(np.float32)


# revision 39
# speedup vs baseline: 1.1429x; 1.0088x over previous
"""Trainium2 Bass kernel for per-query-pair attention (GNN message passing).

Math (reference):
  q = query @ Wq.T + bq                          [B,N,E]
  k = keys @ Wk.T + bk ; v = keys @ Wv.T + bv    [B,N,N,E]
  scores[b,h,i,j] = <k_h[b,i,j], q_h[b,i]> / sqrt(D); probs = softmax_j
  ctx[b,h,i,:]    = sum_j probs * v_h[b,i,j]

Key algebraic collapse (avoids projecting the 128MB keys tensor):
  scores[b,h,i,j] = <keys[b,i,j,:], qk[b,i,h,:]>  with
      qk[b,i,h,:]  = Wk_h.T @ (Wq_h @ query[b,i] + bq_h) / sqrt(D)   (tiny)
  bk drops out of softmax (constant over j).
  ctx[b,h,i,:]    = Wv_h @ u[b,i,h,:] + bv_h      with
      u[b,i,h,:]  = sum_j probs[b,h,i,j] * keys[b,i,j,:]
  (bv passes through since sum_j probs = 1.)

The big tensor is only ever contracted raw: once over e (scores) and once
over j (context weights). Keys ship in fp8 e3m4 (4 mantissa bits; keys are
N(0,1) so a x2 scale sits well inside e3m4's +-15.5 range) in both layouts
([j,i,e] natural and [e,i,j] transposed) - 8.4MB/core, half the bf16 cost,
measured rel-err ~1.7e-2 vs the 2e-2 gate. The x2 key scale is undone on
host: qk carries /2 (scores exact) and Wv carries /2 (context exact).

All key data streams on ONE DMA ring (sync) in exact consumption order
(kst_c, ksn_c per chunk), issued up front: two rings share the 16 DMA
engines in enqueue order and starve each other; the single ring sustains
~390GB/s. Per 16-query chunk: scores (4-query col-group packed matmuls,
8-col qk stationaries) -> exp (scalar) -> reduce/recip/normalize (DVE) ->
PE transpose -> u matmuls (fp8 128-col stationaries, ~27ns/MM pipelined)
-> per-chunk Wv projection into a transposed f32 accumulator (psum->sbuf
copies on scalar). Three-stage software pipeline - scores(c) |
transpose(c-2) | u+proj(c-3) - keeps the PE's in-order queue fed while a
chunk's softmax chain (~3us of cross-engine latency) completes; bias is
applied once at the end on the o-partition accumulator.

Sharding: data-parallel over B (8 batches over 8 cores), zero collectives.
"""

import math

import numpy as np
import ml_dtypes

B, N, E, H, D = 8, 128, 256, 8, 32
NCORES = 8
NCH = 8                      # chunks of 16 queries
GC = 16
NG4 = GC // 4                # score groups (of 4 queries) per chunk
BF16 = ml_dtypes.bfloat16
F8E3 = ml_dtypes.float8_e3m4
KSCALE = np.float32(2.0)     # keys quantize as 2k in e3m4; qk and Wv carry /2

_CACHE = {}


def _enable_ldw_opt():
    """Flip walrus's hardcoded --enable-ldw-opt=false to true (fast weight
    load) when CC_LDW_OPT=1, for A/B testing the u-matmul weight-load rate."""
    import os

    if os.environ.get("CC_LDW_OPT", "0") != "1":
        return
    from concourse import bass_utils

    if getattr(bass_utils, "_ldw_patched", False):
        return
    orig = bass_utils.run_command

    def patched(argv, **kw):
        argv = [
            "--enable-ldw-opt=true" if a == "--enable-ldw-opt=false" else a
            for a in argv
        ]
        return orig(argv, **kw)

    bass_utils.run_command = patched
    bass_utils._ldw_patched = True


def _build_bass():
    import concourse.bass as bass  # noqa: F401
    import concourse.mybir as mybir
    from concourse import bacc
    import concourse.tile as tile
    from concourse.masks import make_identity

    dt = mybir.dt
    fp32 = dt.float32
    bf16 = dt.bfloat16
    fp8 = dt.float8e3

    nc = bacc.Bacc()

    # [j, i, e] fp8 - natural layout, j on partitions
    ks_nat = nc.declare_dram_parameter("ks_nat", [N, N, E], fp8, isOutput=False)
    # [half, e_half, i, j] fp8 - transposed layout, e on partitions
    ks_t = nc.declare_dram_parameter("ks_t", [2, 128, N, N], fp8, isOutput=False)
    # [half, e_half, i, h] bf16 - per-query qk vectors (carry the /2 key scale)
    qk = nc.declare_dram_parameter("qk", [2, 128, N, H], bf16, isOutput=False)
    # [half, e_half, e_out] bf16 - Wv.T / 2
    wvt = nc.declare_dram_parameter("wvt", [2, 128, E], bf16, isOutput=False)
    # [p, half] f32 - bv rearranged so partition p = e_out % 128
    bvp = nc.declare_dram_parameter("bvp", [128, 2], fp32, isOutput=False)
    out = nc.declare_dram_parameter("out", [N, E], fp32, isOutput=True)

    with tile.TileContext(nc) as tc:
        with (
            tc.tile_pool(name="const", bufs=1) as const,
            tc.tile_pool(name="ksn", bufs=1) as ksn_pool,
            tc.tile_pool(name="kst", bufs=1) as kst_pool,
            tc.tile_pool(name="work", bufs=5) as work,
            tc.tile_pool(name="ps_sc", bufs=3, space="PSUM") as ps_sc,
            tc.tile_pool(name="ps_pt", bufs=2, space="PSUM") as ps_pt,
            tc.tile_pool(name="ps_u", bufs=2, space="PSUM") as ps_u,
            tc.tile_pool(name="ps_pj", bufs=1, space="PSUM") as ps_pj,
        ):
            ident_bf = const.tile([128, 128], bf16, tag="ident_bf")
            make_identity(nc, ident_bf)
            ident_f32 = const.tile([128, 128], fp32, tag="ident_f32")
            make_identity(nc, ident_f32)
            # transposed output accumulator [e_out%128, hg, i] f32
            osbT = const.tile([128, 2, N], fp32, tag="osbT")

            # qk for all queries upfront on the sync ring (chunk-0 scores
            # need it); keys stream per-chunk on two rings: sync=ksn,
            # scalar=kst. wvt/bv ride the lighter scalar ring early.
            # ONE DMA ring (sync) for all key data, issued up front in exact
            # consumption order: kst_c right before ksn_c, chunk by chunk.
            # Two competing rings share the 16 DMA engines in rough enqueue
            # order, so whichever ring queues first starves the other for
            # milliseconds-relevant stretches (measured: kst crawled at
            # 43-92GB/s behind a flooded ksn ring while scores starved).
            # A single ring sustains ~330GB/s and completes granules in the
            # exact order compute consumes them. Sem-lane recycle waits on
            # the 17th+ issue stall only the sync FIFO, which has no compute.
            # wvt/bv ride scalar before any compute is queued there.
            wvt_sb = const.tile([128, 2, E], bf16, tag="wvt_sb")
            nc.scalar.dma_start(out=wvt_sb, in_=wvt.rearrange("h e o -> e h o"))
            bv_sb = const.tile([128, 2], fp32, tag="bv_sb")
            nc.scalar.dma_start(out=bv_sb, in_=bvp[:, :])
            qk_sb = const.tile([128, 2, N, H], bf16, tag="qk_sb")
            nc.sync.dma_start(out=qk_sb, in_=qk.rearrange("h e i k -> e h i k"))

            kst_tiles = []
            ksn_tiles = []
            for c in range(NCH):
                i0 = c * GC
                kst = kst_pool.tile([128, 2, GC, N], fp8, tag=f"kst{c}")
                nc.sync.dma_start(
                    out=kst,
                    in_=ks_t[:, :, i0 : i0 + GC, :].rearrange("h e i j -> e h i j"),
                )
                kst_tiles.append(kst)
                ksn = ksn_pool.tile([128, GC, E], fp8, tag=f"ksn{c}")
                nc.sync.dma_start(out=ksn, in_=ks_nat[:, i0 : i0 + GC, :])
                ksn_tiles.append(ksn)

            def front(c):
                """Scores + softmax for chunk c. Returns state for back()."""
                i0 = c * GC
                kst = kst_tiles[c]

                # scores: sc[32*gi+h, g4, j] = sum_e qk[e,i,h] * kst[e,i,j]
                # matmuls write 8 of every 32 rows; zero first so the batched
                # softmax reads only finite values
                sc = ps_sc.tile([128, NG4, N], fp32, tag="sc")
                nc.vector.memset(sc, 0.0)
                for g4 in range(NG4):
                    for gi in range(4):
                        il = g4 * 4 + gi
                        for half in range(2):
                            nc.tensor.matmul(
                                sc[32 * gi : 32 * gi + H, g4, :],
                                lhsT=qk_sb[:, half, i0 + il, :],
                                rhs=kst[:, half, il, :],
                                start=(half == 0),
                                stop=(half == 1),
                                tile_position=(0, 32 * gi),
                            )

                # softmax over j (no max-sub: |scores| < ~6); exp on scalar,
                # everything else on DVE. The scalar FIFO otherwise only
                # holds PE-gated copies from back(), two chunks behind, so
                # exp never queues behind a stalled instruction.
                wsb = work.tile([128, NG4, N], bf16, tag="wsb")
                nc.scalar.activation(
                    out=wsb, in_=sc, func=mybir.ActivationFunctionType.Exp
                )
                zsb = work.tile([128, NG4], fp32, tag="zsb")
                nc.vector.reduce_sum(zsb, wsb, axis=mybir.AxisListType.X)
                rz = work.tile([128, NG4], fp32, tag="rz")
                nc.vector.reciprocal(rz, zsb)
                probs = work.tile([128, NG4, N], bf16, tag="probs")
                for g4 in range(NG4):
                    nc.vector.tensor_scalar_mul(
                        probs[:, g4, :], wsb[:, g4, :], rz[:, g4 : g4 + 1]
                    )
                return c, probs

            def back_t(state):
                """transpose probs -> [j, (gi, h)] and copy to SBUF."""
                c, probs = state
                pt = ps_pt.tile([128, NG4, 128], bf16, tag="pt")
                for g4 in range(NG4):
                    nc.tensor.transpose(pt[:, g4, :], probs[:, g4, :], ident_bf)
                ptsb = work.tile([128, NG4, 128], bf16, tag="ptsb")
                nc.vector.tensor_copy(ptsb, pt)
                return c, ptsb

            def back_u(state):
                """u + projection for a chunk whose probsT landed in SBUF.

                Runs one chunk behind back_t (and two behind front) so the
                PE's in-order queue never waits on the transpose->SBUF copy
                round trip: by the time u's matmuls pop, ptsb has been in
                SBUF for a full pipeline stage.
                """
                c, ptsb = state
                i0 = c * GC
                ksn = ksn_tiles[c]

                # u[e, half, i, h] = sum_j ksn[j, i, e] * probsT[j, (i,h)]
                ups = ps_u.tile([128, 2, GC, H], fp32, tag="ups")
                for g4 in range(NG4):
                    for gi in range(4):
                        il = g4 * 4 + gi
                        for half in range(2):
                            nc.tensor.matmul(
                                ups[:, half, il, :],
                                lhsT=ksn[:, il, 128 * half : 128 * (half + 1)],
                                rhs=ptsb[:, g4, 32 * gi : 32 * gi + H],
                                start=True,
                                stop=True,
                            )
                uc = work.tile([128, 2, GC, H], bf16, tag="uc")
                nc.scalar.copy(out=uc, in_=ups)

                # per-chunk Wv projection into osbT (hides the tail)
                # cps[32*hh+d, i] = sum_e Wv[h*32+d, e]/2 * u[e, i, h]
                # bias is added once at the very end (osbT has e_out on
                # partitions), keeping this copy off the DVE critical path
                for hg in range(2):
                    # bank-sized (512 f32 = 2KB) so the accumulation groups
                    # own their psum zero region exclusively
                    cpsb = ps_pj.tile([128, 512], fp32, tag="pj")
                    cps = cpsb[:, :GC]
                    for hh in range(4):
                        h = hg * 4 + hh
                        for half in range(2):
                            nc.tensor.matmul(
                                cps[32 * hh : 32 * hh + 32, :],
                                lhsT=wvt_sb[:, half, 32 * h : 32 * (h + 1)],
                                rhs=uc[:, half, :, h],
                                start=(half == 0),
                                stop=(half == 1),
                                tile_position=(0, 32 * hh),
                            )
                    nc.scalar.copy(out=osbT[:, hg, i0 : i0 + GC], in_=cps)

            # three-stage software pipeline, each stage chunks deeper:
            # u+proj(c-4) | scores(c) | transposes(c-2). back_u is emitted
            # FIRST in each iteration so uc/proj never queue on the scalar
            # FIFO behind a future chunk's exp; the PE always has future
            # scores queued while a softmax completes, and consumes each
            # ptsb well after the DVE copied it.
            fr, tr = [], []
            for c in range(NCH):
                fr.append(front(c))
                if len(fr) > 2:
                    tr.append(back_t(fr.pop(0)))
                if len(tr) > 1:
                    back_u(tr.pop(0))
            for s in fr:
                tr.append(back_t(s))
            for s in tr:
                back_u(s)

            # ---- tail: add bias (per-partition on osbT), transpose
            # [e_out, i] -> [i, e_out] and write out ----
            osb = const.tile([128, E], fp32, tag="osb")
            for hg in range(2):
                nc.vector.tensor_scalar_add(
                    osbT[:, hg, :], osbT[:, hg, :], bv_sb[:, hg : hg + 1]
                )
                ops = ps_pj.tile([128, 128], fp32, tag="pj")
                nc.tensor.transpose(ops, osbT[:, hg, :], ident_f32)
                nc.vector.tensor_copy(osb[:, 128 * hg : 128 * (hg + 1)], ops)

            nc.sync.dma_start(out=out[:, :], in_=osb)

    nc.finalize()
    return nc


def _host_prep(query_states, key_states, Wq, bq, Wk, bk, Wv, bv):
    """Build per-core input maps. bk is softmax-invariant and dropped."""
    f32 = np.float32
    qs = np.asarray(query_states, f32)
    ks = np.asarray(key_states, f32)
    Wq = np.asarray(Wq, f32)
    bq = np.asarray(bq, f32)
    Wk = np.asarray(Wk, f32)
    Wv = np.asarray(Wv, f32)
    bv = np.asarray(bv, f32)

    q = qs @ Wq.T + bq                                   # [B,N,E]
    qk = np.einsum(
        "bihd,hde->bihe", q.reshape(B, N, H, D), Wk.reshape(H, D, E)
    ) * f32(1.0 / math.sqrt(D) / KSCALE)                 # [B,N,H,E]

    wvt_host = np.ascontiguousarray((Wv.T / KSCALE).reshape(2, 128, E)).astype(BF16)
    bv_host = np.ascontiguousarray(bv.reshape(2, 128).T)

    in_maps = []
    for b in range(B):
        ksb = ks[b] * KSCALE
        in_maps.append(
            {
                "ks_nat": np.ascontiguousarray(ksb.transpose(1, 0, 2)).astype(F8E3),
                "ks_t": np.ascontiguousarray(ksb.transpose(2, 0, 1))
                .reshape(2, 128, N, N)
                .astype(F8E3),
                "qk": np.ascontiguousarray(qk[b].transpose(2, 0, 1))
                .reshape(2, 128, N, H)
                .astype(BF16),
                "wvt": wvt_host,
                "bvp": bv_host,
            }
        )
    return in_maps


def kernel(**inputs):
    _enable_ldw_opt()
    from concourse.bass_utils import run_bass_kernel_spmd

    if "nc" not in _CACHE:
        _CACHE["nc"] = _build_bass()
    nc = _CACHE["nc"]

    in_maps = _host_prep(**inputs)
    res = run_bass_kernel_spmd(nc, in_maps, core_ids=list(range(NCORES)))
    out = np.stack([r["out"] for r in res.results], axis=0)  # [B, N, E]
    return out.astype(np.float32)


# revision 40
# speedup vs baseline: 1.1632x; 1.0177x over previous
"""Trainium2 Bass kernel for per-query-pair attention (GNN message passing).

Math (reference):
  q = query @ Wq.T + bq                          [B,N,E]
  k = keys @ Wk.T + bk ; v = keys @ Wv.T + bv    [B,N,N,E]
  scores[b,h,i,j] = <k_h[b,i,j], q_h[b,i]> / sqrt(D); probs = softmax_j
  ctx[b,h,i,:]    = sum_j probs * v_h[b,i,j]

Key algebraic collapse (avoids projecting the 128MB keys tensor):
  scores[b,h,i,j] = <keys[b,i,j,:], qk[b,i,h,:]>  with
      qk[b,i,h,:]  = Wk_h.T @ (Wq_h @ query[b,i] + bq_h) / sqrt(D)   (tiny)
  bk drops out of softmax (constant over j).
  ctx[b,h,i,:]    = Wv_h @ u[b,i,h,:] + bv_h      with
      u[b,i,h,:]  = sum_j probs[b,h,i,j] * keys[b,i,j,:]
  (bv passes through since sum_j probs = 1.)

The big tensor is only ever contracted raw: once over e (scores) and once
over j (context weights). Keys ship in fp8 e3m4 (4 mantissa bits; keys are
N(0,1) so a x2 scale sits well inside e3m4's +-15.5 range) in both layouts
([j,i,e] natural and [e,i,j] transposed) - 8.4MB/core, half the bf16 cost,
measured rel-err ~1.7e-2 vs the 2e-2 gate. The x2 key scale is undone on
host: qk carries /2 (scores exact) and Wv carries /2 (context exact).

All key data streams on ONE DMA ring (sync) in exact consumption order
(kst_c, ksn_c per chunk), issued up front: two rings share the 16 DMA
engines in enqueue order and starve each other; the single ring sustains
~390GB/s. Per 16-query chunk: scores (4-query col-group packed matmuls,
8-col qk stationaries) -> exp (scalar) -> reduce/recip/normalize (DVE) ->
PE transpose -> u matmuls (fp8 128-col stationaries, ~27ns/MM pipelined)
-> per-chunk Wv projection into a transposed f32 accumulator (psum->sbuf
copies on scalar). Three-stage software pipeline - scores(c) |
transpose(c-2) | u+proj(c-3) - keeps the PE's in-order queue fed while a
chunk's softmax chain (~3us of cross-engine latency) completes; bias is
applied once at the end on the o-partition accumulator.

Sharding: data-parallel over B (8 batches over 8 cores), zero collectives.
"""

import math

import numpy as np
import ml_dtypes

B, N, E, H, D = 8, 128, 256, 8, 32
NCORES = 8
NCH = 8                      # chunks of 16 queries
GC = 16
NG4 = GC // 4                # score groups (of 4 queries) per chunk
BF16 = ml_dtypes.bfloat16
F8E3 = ml_dtypes.float8_e3m4
KSCALE = np.float32(2.0)     # keys quantize as 2k in e3m4; qk and Wv carry /2

_CACHE = {}


def _enable_ldw_opt():
    """Flip walrus's hardcoded --enable-ldw-opt=false to true (fast weight
    load) when CC_LDW_OPT=1, for A/B testing the u-matmul weight-load rate."""
    import os

    if os.environ.get("CC_LDW_OPT", "0") != "1":
        return
    from concourse import bass_utils

    if getattr(bass_utils, "_ldw_patched", False):
        return
    orig = bass_utils.run_command

    def patched(argv, **kw):
        argv = [
            "--enable-ldw-opt=true" if a == "--enable-ldw-opt=false" else a
            for a in argv
        ]
        return orig(argv, **kw)

    bass_utils.run_command = patched
    bass_utils._ldw_patched = True


def _build_bass():
    import concourse.bass as bass  # noqa: F401
    import concourse.mybir as mybir
    from concourse import bacc
    import concourse.tile as tile
    from concourse.masks import make_identity

    dt = mybir.dt
    fp32 = dt.float32
    bf16 = dt.bfloat16
    fp8 = dt.float8e3

    nc = bacc.Bacc()

    # [j, i, e] fp8 - natural layout, j on partitions
    ks_nat = nc.declare_dram_parameter("ks_nat", [N, N, E], fp8, isOutput=False)
    # [half, e_half, i, j] fp8 - transposed layout, e on partitions
    ks_t = nc.declare_dram_parameter("ks_t", [2, 128, N, N], fp8, isOutput=False)
    # [half, e_half, i, h] bf16 - per-query qk vectors (carry the /2 key scale)
    qk = nc.declare_dram_parameter("qk", [2, 128, N, H], bf16, isOutput=False)
    # [half, e_half, e_out] bf16 - Wv.T / 2
    wvt = nc.declare_dram_parameter("wvt", [2, 128, E], bf16, isOutput=False)
    # [p, half] f32 - bv rearranged so partition p = e_out % 128
    bvp = nc.declare_dram_parameter("bvp", [128, 2], fp32, isOutput=False)
    out = nc.declare_dram_parameter("out", [N, E], fp32, isOutput=True)

    with tile.TileContext(nc) as tc:
        with (
            tc.tile_pool(name="const", bufs=1) as const,
            tc.tile_pool(name="ksn", bufs=1) as ksn_pool,
            tc.tile_pool(name="kst", bufs=1) as kst_pool,
            tc.tile_pool(name="work", bufs=5) as work,
            tc.tile_pool(name="ps_sc", bufs=3, space="PSUM") as ps_sc,
            tc.tile_pool(name="ps_pt", bufs=2, space="PSUM") as ps_pt,
            tc.tile_pool(name="ps_u", bufs=2, space="PSUM") as ps_u,
            tc.tile_pool(name="ps_pj", bufs=1, space="PSUM") as ps_pj,
        ):
            ident_bf = const.tile([128, 128], bf16, tag="ident_bf")
            make_identity(nc, ident_bf)
            ident_f32 = const.tile([128, 128], fp32, tag="ident_f32")
            make_identity(nc, ident_f32)
            # transposed output accumulator [e_out%128, hg, i] f32
            osbT = const.tile([128, 2, N], fp32, tag="osbT")

            # qk for all queries upfront on the sync ring (chunk-0 scores
            # need it); keys stream per-chunk on two rings: sync=ksn,
            # scalar=kst. wvt/bv ride the lighter scalar ring early.
            # ONE DMA ring (sync) for all key data, issued up front in exact
            # consumption order: kst_c right before ksn_c, chunk by chunk.
            # Two competing rings share the 16 DMA engines in rough enqueue
            # order, so whichever ring queues first starves the other for
            # milliseconds-relevant stretches (measured: kst crawled at
            # 43-92GB/s behind a flooded ksn ring while scores starved).
            # A single ring sustains ~330GB/s and completes granules in the
            # exact order compute consumes them. Sem-lane recycle waits on
            # the 17th+ issue stall only the sync FIFO, which has no compute.
            # wvt/bv ride scalar before any compute is queued there.
            wvt_sb = const.tile([128, 2, E], bf16, tag="wvt_sb")
            nc.scalar.dma_start(out=wvt_sb, in_=wvt.rearrange("h e o -> e h o"))
            bv_sb = const.tile([128, 2], fp32, tag="bv_sb")
            nc.scalar.dma_start(out=bv_sb, in_=bvp[:, :])
            # qk split in two tiles: chunk 0/1's slice leads the ring
            # (128KB) so scores start ~2us earlier; the rest rides behind
            # chunk 0's keys. Separate tiles because dependencies are
            # whole-tile.
            qk_r = qk.rearrange("h e i k -> e h i k")
            qk0_sb = const.tile([128, 2, 2 * GC, H], bf16, tag="qk0_sb")
            nc.sync.dma_start(out=qk0_sb, in_=qk_r[:, :, : 2 * GC, :])
            qkr_sb = const.tile([128, 2, N - 2 * GC, H], bf16, tag="qkr_sb")

            def qk_slice(half, i):
                if i < 2 * GC:
                    return qk0_sb[:, half, i, :]
                return qkr_sb[:, half, i - 2 * GC, :]

            kst_tiles = []
            ksn_tiles = []
            for c in range(NCH):
                i0 = c * GC
                kst = kst_pool.tile([128, 2, GC, N], fp8, tag=f"kst{c}")
                nc.sync.dma_start(
                    out=kst,
                    in_=ks_t[:, :, i0 : i0 + GC, :].rearrange("h e i j -> e h i j"),
                )
                kst_tiles.append(kst)
                ksn = ksn_pool.tile([128, GC, E], fp8, tag=f"ksn{c}")
                nc.sync.dma_start(out=ksn, in_=ks_nat[:, i0 : i0 + GC, :])
                ksn_tiles.append(ksn)
                if c == 0:
                    nc.sync.dma_start(
                        out=qkr_sb, in_=qk_r[:, :, 2 * GC :, :]
                    )

            def front(c):
                """Scores + softmax for chunk c. Returns state for back()."""
                i0 = c * GC
                kst = kst_tiles[c]

                # scores: sc[32*gi+h, g4, j] = sum_e qk[e,i,h] * kst[e,i,j]
                # matmuls write 8 of every 32 rows; zero first so the batched
                # softmax reads only finite values
                sc = ps_sc.tile([128, NG4, N], fp32, tag="sc")
                nc.vector.memset(sc, 0.0)
                for g4 in range(NG4):
                    for gi in range(4):
                        il = g4 * 4 + gi
                        for half in range(2):
                            nc.tensor.matmul(
                                sc[32 * gi : 32 * gi + H, g4, :],
                                lhsT=qk_slice(half, i0 + il),
                                rhs=kst[:, half, il, :],
                                start=(half == 0),
                                stop=(half == 1),
                                tile_position=(0, 32 * gi),
                            )

                # softmax over j (no max-sub: |scores| < ~6); exp on scalar,
                # everything else on DVE. The scalar FIFO otherwise only
                # holds PE-gated copies from back(), two chunks behind, so
                # exp never queues behind a stalled instruction.
                wsb = work.tile([128, NG4, N], bf16, tag="wsb")
                nc.scalar.activation(
                    out=wsb, in_=sc, func=mybir.ActivationFunctionType.Exp
                )
                zsb = work.tile([128, NG4], fp32, tag="zsb")
                nc.vector.reduce_sum(zsb, wsb, axis=mybir.AxisListType.X)
                rz = work.tile([128, NG4], fp32, tag="rz")
                nc.vector.reciprocal(rz, zsb)
                probs = work.tile([128, NG4, N], bf16, tag="probs")
                for g4 in range(NG4):
                    nc.vector.tensor_scalar_mul(
                        probs[:, g4, :], wsb[:, g4, :], rz[:, g4 : g4 + 1]
                    )
                return c, probs

            def back_t(state):
                """transpose probs -> [j, (gi, h)] and copy to SBUF."""
                c, probs = state
                pt = ps_pt.tile([128, NG4, 128], bf16, tag="pt")
                for g4 in range(NG4):
                    nc.tensor.transpose(pt[:, g4, :], probs[:, g4, :], ident_bf)
                ptsb = work.tile([128, NG4, 128], bf16, tag="ptsb")
                nc.vector.tensor_copy(ptsb, pt)
                return c, ptsb

            def back_u(state):
                """u + projection for a chunk whose probsT landed in SBUF.

                Runs one chunk behind back_t (and two behind front) so the
                PE's in-order queue never waits on the transpose->SBUF copy
                round trip: by the time u's matmuls pop, ptsb has been in
                SBUF for a full pipeline stage.
                """
                c, ptsb = state
                i0 = c * GC
                ksn = ksn_tiles[c]

                # u[e, half, i, h] = sum_j ksn[j, i, e] * probsT[j, (i,h)]
                ups = ps_u.tile([128, 2, GC, H], fp32, tag="ups")
                for g4 in range(NG4):
                    for gi in range(4):
                        il = g4 * 4 + gi
                        for half in range(2):
                            nc.tensor.matmul(
                                ups[:, half, il, :],
                                lhsT=ksn[:, il, 128 * half : 128 * (half + 1)],
                                rhs=ptsb[:, g4, 32 * gi : 32 * gi + H],
                                start=True,
                                stop=True,
                            )
                uc = work.tile([128, 2, GC, H], bf16, tag="uc")
                nc.scalar.copy(out=uc, in_=ups)

                # per-chunk Wv projection into osbT (hides the tail)
                # cps[32*hh+d, i] = sum_e Wv[h*32+d, e]/2 * u[e, i, h]
                # bias is added once at the very end (osbT has e_out on
                # partitions), keeping this copy off the DVE critical path
                for hg in range(2):
                    # bank-sized (512 f32 = 2KB) so the accumulation groups
                    # own their psum zero region exclusively
                    cpsb = ps_pj.tile([128, 512], fp32, tag="pj")
                    cps = cpsb[:, :GC]
                    for hh in range(4):
                        h = hg * 4 + hh
                        for half in range(2):
                            nc.tensor.matmul(
                                cps[32 * hh : 32 * hh + 32, :],
                                lhsT=wvt_sb[:, half, 32 * h : 32 * (h + 1)],
                                rhs=uc[:, half, :, h],
                                start=(half == 0),
                                stop=(half == 1),
                                tile_position=(0, 32 * hh),
                            )
                    nc.scalar.copy(out=osbT[:, hg, i0 : i0 + GC], in_=cps)

            # three-stage software pipeline, each stage chunks deeper:
            # u+proj(c-4) | scores(c) | transposes(c-2). back_u is emitted
            # FIRST in each iteration so uc/proj never queue on the scalar
            # FIFO behind a future chunk's exp; the PE always has future
            # scores queued while a softmax completes, and consumes each
            # ptsb well after the DVE copied it.
            fr, tr = [], []
            for c in range(NCH):
                if len(fr) >= 2:
                    tr.append(back_t(fr.pop(0)))
                if len(tr) > 1:
                    back_u(tr.pop(0))
                fr.append(front(c))
            while fr:
                tr.append(back_t(fr.pop(0)))
                if len(tr) > 1:
                    back_u(tr.pop(0))
            while tr:
                back_u(tr.pop(0))

            # ---- tail: add bias (per-partition on osbT), transpose
            # [e_out, i] -> [i, e_out] and write out ----
            osb = const.tile([128, E], fp32, tag="osb")
            for hg in range(2):
                nc.vector.tensor_scalar_add(
                    osbT[:, hg, :], osbT[:, hg, :], bv_sb[:, hg : hg + 1]
                )
                ops = ps_pj.tile([128, 128], fp32, tag="pj")
                nc.tensor.transpose(ops, osbT[:, hg, :], ident_f32)
                nc.vector.tensor_copy(osb[:, 128 * hg : 128 * (hg + 1)], ops)

            nc.sync.dma_start(out=out[:, :], in_=osb)

    nc.finalize()
    return nc


def _host_prep(query_states, key_states, Wq, bq, Wk, bk, Wv, bv):
    """Build per-core input maps. bk is softmax-invariant and dropped."""
    f32 = np.float32
    qs = np.asarray(query_states, f32)
    ks = np.asarray(key_states, f32)
    Wq = np.asarray(Wq, f32)
    bq = np.asarray(bq, f32)
    Wk = np.asarray(Wk, f32)
    Wv = np.asarray(Wv, f32)
    bv = np.asarray(bv, f32)

    q = qs @ Wq.T + bq                                   # [B,N,E]
    qk = np.einsum(
        "bihd,hde->bihe", q.reshape(B, N, H, D), Wk.reshape(H, D, E)
    ) * f32(1.0 / math.sqrt(D) / KSCALE)                 # [B,N,H,E]

    wvt_host = np.ascontiguousarray((Wv.T / KSCALE).reshape(2, 128, E)).astype(BF16)
    bv_host = np.ascontiguousarray(bv.reshape(2, 128).T)

    in_maps = []
    for b in range(B):
        ksb = ks[b] * KSCALE
        in_maps.append(
            {
                "ks_nat": np.ascontiguousarray(ksb.transpose(1, 0, 2)).astype(F8E3),
                "ks_t": np.ascontiguousarray(ksb.transpose(2, 0, 1))
                .reshape(2, 128, N, N)
                .astype(F8E3),
                "qk": np.ascontiguousarray(qk[b].transpose(2, 0, 1))
                .reshape(2, 128, N, H)
                .astype(BF16),
                "wvt": wvt_host,
                "bvp": bv_host,
            }
        )
    return in_maps


def kernel(**inputs):
    _enable_ldw_opt()
    from concourse.bass_utils import run_bass_kernel_spmd

    if "nc" not in _CACHE:
        _CACHE["nc"] = _build_bass()
    nc = _CACHE["nc"]

    in_maps = _host_prep(**inputs)
    res = run_bass_kernel_spmd(nc, in_maps, core_ids=list(range(NCORES)))
    out = np.stack([r["out"] for r in res.results], axis=0)  # [B, N, E]
    return out.astype(np.float32)
